# revision 32
# baseline (speedup 1.0000x reference)
"""LIF spike kernel for Trainium2 (Bass/Tile), 8-core data-parallel.

Problem: x [B=32, C=128, H=32, W=32, T=8] f32; LIF membrane scan over T:
    u_t = TAU * u_{t-1} * (1 - o_{t-1}) + x_t;   o_t = (u_t - VTH > 0)
Output: o [B, C, H, W, T] f32 (values are exactly 0.0 / 1.0).

Strategy (v4):
  - Shard batch (32 -> 4 per core), no cross-core communication.
  - Per core the data is a flat [128, 32768] f32 block in DRAM, native
    (time-fastest, "interleaved") layout; tiles of E sites scanned over t.
  - Sites are partitioned between two fully decoupled pipelines (no
    cross-engine edges inside either recurrence, so the tile scheduler
    cannot stall one engine on the other):
    DVE pipeline (sites [0, GPS_OFF)):
        u_t = STT(s, TAU, x, mult, add);  s_t = STT(u, VTH, u, is_le, mult)
        ACT: sgn_t = Sign(u_t - VTH) -> bf16; PE: pack += (4^t I) @ sgn_t
        (exact: products +-4^t, integer sums < 2^15); PSUM -> int16 out.
    GPSIMD pipeline (sites [GPS_OFF, 4096)):
        m_t = TS(u, VTH, is_le) -> int8 {0,1} written strided into the
        output tile (dual use: output digit AND mask operand);
        s_t = TT(m_t, u, mult); u_{t+1} = TS(s, TAU, mult)+TT(+x).
  - Outputs: o_pk int16 balanced-base-4 packed (DVE sites, 2B/site),
    o_gps int8 per (site, t) (GPS sites). Host decodes: DVE bit =
    (base-4 digit == 2), GPS bit = (m == 0). Bit-exact vs the fp32
    reference: mask-multiply is exact; (u - 0.3 > 0) <=> (u > 0.3) in
    fp32; u == VTH maps to digit 0 / m=1 -> no spike, as the reference.
"""

import numpy as np

TAU = 0.2
VTH = 0.3

B, C, H, W, T = 32, 128, 32, 32, 8
NCORES = 8
P = 128
SHARD_B = B // NCORES                  # 4 batches per core
VALS = SHARD_B * C * H * W * T         # 4_194_304 values per core
COLS = VALS // P                       # 32768 per partition row
SITES = COLS // T                      # 4096 sites per partition row
BIAS4 = (4 ** T - 1) // 3              # 21845: balanced-base-4 offset

# Tunables: per-slot tile size lists; DVE slots then GPS slots own
# consecutive site ranges.
DVE_SLOTS = ((128, 512, 512), (448, 512), (512, 256, 256))
GPS_SLOTS = ((448, 512),)
GPS_SITES = sum(sum(s) for s in GPS_SLOTS)
GPS_OFF = SITES - GPS_SITES
IO_BUFS = 2
TMP_BUFS = 3
O_BUFS = 2
COPY_ENGINE = "act"

_cached = None


def _make_w():
    import ml_dtypes
    w = np.zeros((P, T * P), dtype=np.float32)
    idx = np.arange(P)
    for t in range(T):
        w[idx, t * P + idx] = np.float32(4.0 ** t)
    return w.astype(ml_dtypes.bfloat16)


def _build_nc(dve_slots=DVE_SLOTS, gps_slots=GPS_SLOTS, io_bufs=IO_BUFS,
              tmp_bufs=TMP_BUFS, o_bufs=O_BUFS, copy_engine=COPY_ENGINE):
    import concourse.bass as bass
    import concourse.bacc as bacc
    import concourse.tile as tile
    from concourse import mybir

    f32 = mybir.dt.float32
    bf16 = mybir.dt.bfloat16
    i16 = mybir.dt.int16
    i8 = mybir.dt.int8
    Alu = mybir.AluOpType
    Act = mybir.ActivationFunctionType

    gps_sites = sum(sum(s) for s in gps_slots)
    gps_off = SITES - gps_sites

    nc = bacc.Bacc("TRN2", target_bir_lowering=False, debug=False)
    x = nc.dram_tensor("x", [P, COLS], f32, kind="ExternalInput")
    w = nc.dram_tensor("w", [P, T * P], bf16, kind="ExternalInput")
    o_pk = nc.dram_tensor("o_pk", [P, max(gps_off, 1)], i16,
                          kind="ExternalOutput")
    o_gps = nc.dram_tensor("o_gps", [P, max(gps_sites * T, 1)], i8,
                           kind="ExternalOutput")

    # slot spec: (engine, [sizes]); first DVE slot leads (small first tile
    # so DVE starts fast), then GPS slots so Pool's DMAs stay near the
    # head of the queue.
    specs = [("dve", list(s)) for s in dve_slots]
    gspecs = [("gps", list(s)) for s in gps_slots]
    order = specs[:1] + gspecs + specs[1:]

    with tile.TileContext(nc) as tc:
        with (
            tc.tile_pool(name="const", bufs=1) as cpool,
            tc.tile_pool(name="io", bufs=io_bufs) as io_pool,
            tc.tile_pool(name="out", bufs=2) as out_pool,
            tc.tile_pool(name="tmp", bufs=tmp_bufs) as tmp_pool,
            tc.tile_pool(name="opool", bufs=o_bufs) as o_pool,
            tc.tile_pool(name="psum", bufs=2, space="PSUM") as pp,
        ):
            neg_vth = cpool.tile([P, 1], f32, tag="neg_vth")
            nc.vector.memset(neg_vth[:], -VTH)
            wt = cpool.tile([P, T * P], bf16, tag="w")
            nc.sync.dma_start(wt[:], w[:, :])
            if gps_sites == 0:
                # keep the (otherwise unwritten) o_gps output legal
                zi = cpool.tile([P, 1], i8, tag="zi")
                nc.vector.memset(zi[:], 0)
                nc.sync.dma_start(o_gps[:, 0:1], zi[:])

            # assign site offsets: DVE slots pack [0, gps_off), GPS slots
            # pack [gps_off, SITES)
            doff, goff = 0, gps_off
            st = []
            for eng, sizes in order:
                q = []
                for sz in sizes:
                    if eng == "dve":
                        q.append((doff, sz))
                        doff += sz
                    else:
                        q.append((goff, sz))
                        goff += sz
                st.append({"eng": eng, "q": q, "j": None, "t": 0,
                           "xr": None, "u": None, "s": None, "pk": None,
                           "og": None, "sz": 0, "off": 0})
            assert doff == gps_off and goff == SITES
            K = len(st)

            def issue_dma(k):
                s = st[k]
                toff, tsz = s["q"].pop(0)
                xin = io_pool.tile([P, 512 * T], f32, tag=f"xin{k}")
                nc.sync.dma_start(
                    xin[:, : tsz * T], x[:, toff * T : (toff + tsz) * T]
                )
                xr = xin[:, : tsz * T].rearrange("p (e t) -> p e t", t=T)
                return (toff, tsz), xr

            pending = {}
            for k in range(K):
                if st[k]["q"]:
                    pending[k] = issue_dma(k)

            def work_left():
                return any(
                    s["j"] is not None or k in pending
                    for k, s in enumerate(st)
                )

            vt = [0.0] * K

            def step_cost(eng, sz, t):
                per = sz / 512.0
                n = 1 if t in (0, T - 1) else 2
                return per * (1965.0 if eng == "gps" else 595.0) * n

            while work_left():
                cand = [
                    k for k, s in enumerate(st)
                    if s["j"] is not None or k in pending
                ]
                if not cand:
                    break
                k = min(cand, key=lambda k: vt[k])
                s = st[k]
                if s["j"] is None:
                    (s["off"], s["sz"]), s["xr"] = pending.pop(k)
                    s["j"], s["t"] = True, 0
                    if s["q"]:
                        # issue next tile's DMA one tile ahead (io_bufs=2)
                        pending[k] = issue_dma(k)
                t, sz, eng = s["t"], s["sz"], s["eng"]
                vt[k] += step_cost(eng, sz, t)

                if eng == "gps":
                    # u_t
                    if t == 0:
                        s["u"] = s["xr"][:, :, 0]
                    else:
                        g = tmp_pool.tile([P, 512], f32, tag=f"g{k}")
                        nc.gpsimd.tensor_scalar(
                            g[:, :sz], s["s"], TAU, None, Alu.mult
                        )
                        u = tmp_pool.tile([P, 512], f32, tag=f"u{k}")
                        nc.gpsimd.tensor_tensor(
                            u[:, :sz], g[:, :sz], s["xr"][:, :, t], Alu.add
                        )
                        s["u"] = u[:, :sz]
                    if t == 0:
                        og = out_pool.tile([P, 512 * T], i8, tag=f"og{k}")
                        s["og"] = og
                    ogr = s["og"][:, : sz * T].rearrange(
                        "p (e t) -> p e t", t=T
                    )
                    # m_t -> int8 {0,1} strided into output tile
                    nc.gpsimd.tensor_scalar(
                        ogr[:, :, t], s["u"], VTH, None, Alu.is_le
                    )
                    if t < T - 1:
                        sn = tmp_pool.tile([P, 512], f32, tag=f"s{k}")
                        nc.gpsimd.tensor_tensor(
                            sn[:, :sz], ogr[:, :, t], s["u"], Alu.mult
                        )
                        s["s"] = sn[:, :sz]
                        s["t"] += 1
                    else:
                        toff = s["off"] - gps_off
                        nc.sync.dma_start(
                            o_gps[:, toff * T : (toff + sz) * T],
                            s["og"][:, : sz * T],
                        )
                        s["j"] = None
                    continue

                # DVE pipeline
                if t == 0:
                    s["u"] = s["xr"][:, :, 0]
                else:
                    u = tmp_pool.tile([P, 512], f32, tag=f"u{k}")
                    nc.vector.scalar_tensor_tensor(
                        u[:, :sz], s["s"], TAU, s["xr"][:, :, t],
                        Alu.mult, Alu.add,
                    )
                    s["u"] = u[:, :sz]
                sg = o_pool.tile([P, 512], bf16, tag=f"o{k}")
                nc.scalar.activation(
                    sg[:, :sz], s["u"], Act.Sign, bias=neg_vth[:], scale=1.0
                )
                if t == 0:
                    pk = pp.tile([P, 512], f32, tag=f"pk{k}")
                    s["pk"] = pk
                nc.tensor.matmul(
                    s["pk"][:, :sz], wt[:, t * P : (t + 1) * P], sg[:, :sz],
                    start=(t == 0), stop=(t == T - 1),
                )
                if t < T - 1:
                    sn = tmp_pool.tile([P, 512], f32, tag=f"s{k}")
                    nc.vector.scalar_tensor_tensor(
                        sn[:, :sz], s["u"], VTH, s["u"], Alu.is_le, Alu.mult
                    )
                    s["s"] = sn[:, :sz]
                    s["t"] += 1
                else:
                    oi = out_pool.tile([P, 512], i16, tag=f"out{k}")
                    if copy_engine == "act":
                        nc.scalar.copy(oi[:, :sz], s["pk"][:, :sz])
                    else:
                        nc.vector.tensor_copy(oi[:, :sz], s["pk"][:, :sz])
                    nc.sync.dma_start(
                        o_pk[:, s["off"] : s["off"] + sz], oi[:, :sz]
                    )
                    s["j"] = None
    nc.finalize()
    return nc


FALLBACK_CFGS = [
    {},
    {"copy_engine": "dve"},
    {"dve_slots": ((256, 512, 512, 256), (256, 512, 512, 256),
                   (512, 512)), "gps_slots": ()},
]


def kernel(x):
    global _cached
    from concourse.bass_utils import run_bass_kernel_spmd

    xs = np.ascontiguousarray(np.asarray(x, dtype=np.float32))
    assert xs.shape == (B, C, H, W, T)
    xs = xs.reshape(NCORES, P, COLS)
    wnp = _make_w()
    in_maps = [{"x": xs[i], "w": wnp} for i in range(NCORES)]

    last_err = None
    for cfg in FALLBACK_CFGS:
        try:
            if _cached is None:
                _cached = (_build_nc(**cfg), cfg)
            res = run_bass_kernel_spmd(_cached[0], in_maps,
                                       list(range(NCORES)))
            cfg = _cached[1]
            break
        except Exception as e:
            last_err = e
            _cached = None
    else:
        raise last_err

    gps_slots = cfg.get("gps_slots", GPS_SLOTS)
    gps_sites = sum(sum(s) for s in gps_slots)
    gps_off = SITES - gps_sites

    bits = np.empty((NCORES, P, SITES, T), dtype=bool)
    pk = np.stack([res.results[i]["o_pk"] for i in range(NCORES)])
    q = pk[:, :, :gps_off].astype(np.int32) + BIAS4
    shifts = (2 * np.arange(T, dtype=np.int32))[None, None, None, :]
    bits[:, :, :gps_off] = ((q[..., None] >> shifts) & 3) == 2
    if gps_sites:
        og = np.stack([res.results[i]["o_gps"] for i in range(NCORES)])
        bits[:, :, gps_off:] = (
            og[:, :, : gps_sites * T].reshape(NCORES, P, gps_sites, T) == 0
        )
    return bits.reshape(B, C, H, W, T).astype(np.float32)


if __name__ == "__main__":
    rng = np.random.default_rng(0)
    x = rng.standard_normal((B, C, H, W, T), dtype=np.float32)
    out = kernel(x)
    print("out", out.shape, out.dtype, "spike rate", out.mean())
